# revision 27
# baseline (speedup 1.0000x reference)
"""Self-contained Trainium2 Bass kernel for single-head T2T attention.

Problem: x:[8,4096,512], w_qkv:[1536,512], w_proj:[512,512], b_proj:[512]
    qkv = x @ w_qkv.T ; q,k,v split
    attn = softmax(q @ k.T / sqrt(512))
    out  = v + (attn @ v) @ w_proj.T + b_proj

Sharding: data-parallel over batch B=8 across the 8 NeuronCores (one
example per core); weights replicated.  No collectives needed.

Numerics: the output is dominated by the v residual (||attn path|| /
||out|| ~ 0.8%), so everything EXCEPT the v residual runs in fp8e4
with DoubleRow perf mode (2 fp8 MACs per PE cell per cycle).  Measured
end-to-end rel err ~6e-4 vs the 2e-2 gate.  V is computed in fp32r
(fp22 multiply) and kept in fp32 for the residual.

Projection fusion: (attn @ v) @ w_proj.T == attn @ (v @ w_proj.T) and
v @ w_proj.T == x @ (w_proj @ w_v).T, so phase 0 computes the fused
weight wvp = (w_proj @ w_v) once on the PE (w_v arrives from DRAM
already in the [d, c] layout needed as the stationary operand, so its
transposed [c, e] layout costs no extra transposes).  The attention PV
matmul then directly produces the PROJECTED output in natural [n, e]
orientation -- no O^T materialization, no separate proj stage, and no
PSUM-evacuation stall between attention and projection.

Per-core dataflow (N=4096, C=512, P=128, NQ=512 chunks):
  phase 0: PE-transpose w_qkv/w_proj; Q/K weight halves to fp8 [c,f];
      V third to fp32r [c,d]; wvpT = w_v.T @ w_proj.T via 16 fp32r
      matmuls -> fp8 [c,e].
  phase 1 (per chunk): stream x, PE-transpose to x^T (fp32), one DVE
      copy to fp8 x^T.  Q^T,K^T via fp8 DoubleRow -> resident SBUF.
      VW = x @ wvp.T via fp8 DoubleRow -> fp8 [m,e] tiles for PV.
      V via fp32r matmuls -> fp32 V+b_proj residual tile (bias folded
      here with a DVE add).
  phase 2 (per chunk, per m-block PAIR): S^T = K.Q^T with DoubleRow,
      both pair halves into one 2-bank PSUM tile; one ScalarE exp
      (scale fused; scores bounded ~|1.1| so no max-subtraction)
      writes the fp8 P^T pair; PV DoubleRow matmuls contract over both
      m-blocks at once: out[nb] += P^T[:, nb-cols].T @ VW-pair,
      accumulating the projected output [n, e] in PSUM.  Softmax
      denominators: DVE accumulates the fp8 P tiles (consistent with
      the PV numerator quantization), tiny N=1 matmuls reduce over
      partitions into a dead S-PSUM slot, and the normalization is a
      per-partition scalar folded into the final fused DVE op:
      fin = pvout * (1/rowsum) + (v + b_proj), then DMA out.
"""

import numpy as np

import concourse.bass as bass
import concourse.mybir as mybir
from concourse.tile import TileContext
from concourse.masks import make_identity

P = 128
B = 8
N_FULL = 4096
C = 512
F = 3 * C
NQ = 512           # query chunk width
CB = C // P        # 4 contraction sub-blocks of 128
CBP = CB // 2      # 2 DoubleRow pairs for a 512 contraction
SCALE = 1.0 / float(np.sqrt(C))
F32 = mybir.dt.float32
F32R = mybir.dt.float32r
F8 = mybir.dt.float8e4
DR = mybir.MatmulPerfMode.DoubleRow


# ---------------------------------------------------------------------------
# Workaround: this container's walrus build accepts at most one sync wait per
# plain instruction (two for EventSemaphore), but Tile's wait assignment can
# attach several.  Post-pass: move excess waits onto injected same-engine
# NOPs placed immediately before the over-subscribed instruction.
# ---------------------------------------------------------------------------
def _legalize_waits(nc):
    for fn in nc.m.functions:
        for bb in fn.blocks:
            insts = bb.instructions
            out = []
            changed = False
            for inst in insts:
                si = inst.sync_info
                waits = list(si.on_wait) if si and si.on_wait else []
                cap = 2 if isinstance(inst, mybir.InstEventSemaphore) else 1
                if len(waits) > cap:
                    keep = waits[:cap]
                    rest = waits[cap:]
                    for i, w in enumerate(rest):
                        nop = mybir.InstNoOp(
                            name=f"{inst.name}-wspill{i}",
                            ins=[], outs=[], engine=inst.engine)
                        nop.sync_info = mybir.SyncInfo(
                            on_wait=[w], on_update=[])
                        nc.register_instruction(nop, overwrite=True)
                        out.append(nop)
                    si.on_wait = keep
                    changed = True
                out.append(inst)
            if changed:
                insts.clear()
                insts.extend(out)


class _nullctx:
    def __enter__(self):
        return None

    def __exit__(self, *a):
        return False


def build_program(n=N_FULL, mm_dt=F32R, attn_dt=F8, reps=1, hw_loop=0,
                  loop_phase=0):
    """Build the per-core Bass program for one [n, C] example.

    mm_dt/attn_dt kept for test.py signature compatibility; the kernel
    is fixed at fp32r (V path) + fp8 DoubleRow (everything else).
    loop_phase (timing probes only): 0 = hw_loop wraps both phases,
    1 = loop wraps phase 0+1 only (no attention), 2 = phase 0+1 run
    once and the loop wraps phase 2 only, 3 = like 2 but with the
    softmax-denominator DVE chain stripped (S/exp/PV floor probe).
    NOTE: DoubleRowSwInterleave was probed and CRASHES the exec unit
    (NRT_EXEC_UNIT_UNRECOVERABLE) on this hardware -- do not use.
    """
    n_chunks = n // NQ
    mb_total = n // P
    pair_total = mb_total // 2
    nb_total = NQ // P

    nc = bass.Bass("TRN2", target_bir_lowering=False,
                   dynamic_dma_scratch_size=8192)
    x = nc.dram_tensor("x", (n, C), F32, kind="ExternalInput")
    w_qkv = nc.dram_tensor("w_qkv", (F, C), F32, kind="ExternalInput")
    w_proj = nc.dram_tensor("w_proj", (C, C), F32, kind="ExternalInput")
    b_proj = nc.dram_tensor("b_proj", (C,), F32, kind="ExternalInput")
    out = nc.dram_tensor("out", (n, C), F32, kind="ExternalOutput")

    def f32view(ap):
        # fp32r storage is fp32 bits; view as fp32 for non-PE ops
        return ap.bitcast(F32) if ap.dtype == F32R else ap

    def r32(ap):
        return ap.bitcast(F32R) if ap.dtype == F32 else ap

    with TileContext(nc) as tc:
        with tc.tile_pool(name="singles", bufs=1) as singles:
            ident = singles.tile([P, P], F32)
            make_identity(nc, ident)
            ones_f32 = singles.tile([P, 1], F32)
            nc.vector.memset(ones_f32, 1.0)
            bias_bc2 = singles.tile([P, 2, C], F32)  # pair-shaped broadcast
            for j in range(2):
                nc.sync.dma_start(
                    out=bias_bc2[:, j, :],
                    in_=b_proj[:].unsqueeze(0).to_broadcast((P, C)))

            kT8 = singles.tile([P, CB, n], F8)       # K^T: [d, m]
            qT8 = singles.tile([P, CB, n], F8)       # Q^T: [d, n]
            vw8 = singles.tile([P, mb_total, C], F8)  # VW = V @ w_proj.T
            vb32 = singles.tile([P, mb_total, C], F32)  # V + b_proj residual
            wqkT8 = singles.tile([P, CB, 2 * C], F8)    # [c, f] f in [0,1024)
            wvT = singles.tile([P, CB, C], F32R)        # [c, d] V third
            wprojT = singles.tile([P, CB, C], F32R)     # [d, e]
            wvpT8 = singles.tile([P, CB, C], F8)        # [c, e] fused V-proj

            def phase01():
                with tc.tile_pool(name="wload", bufs=8) as wload, \
                     tc.tile_pool(name="wvnat", bufs=4) as wvnat_pool, \
                     tc.tile_pool(name="xnat", bufs=6) as xnat_pool, \
                     tc.tile_pool(name="xT", bufs=2) as xT_pool, \
                     tc.tile_pool(name="xT8", bufs=2) as xT8_pool, \
                     tc.tile_pool(name="tp_psum", bufs=3, space="PSUM") as tp_psum, \
                     tc.tile_pool(name="mm_psum", bufs=2, space="PSUM") as mm_psum:

                    # w_proj first (wvp matmuls below need wprojT)
                    for eb in range(C // P):
                        wnat = wload.tile([P, C], F32, tag="wnat")
                        nc.sync.dma_start(
                            out=wnat, in_=w_proj[eb * P:(eb + 1) * P, :])
                        tp = tp_psum.tile([P, CB, P], F32, tag="tp")
                        for db in range(CB):
                            nc.tensor.transpose(
                                tp[:, db, :], wnat[:, db * P:(db + 1) * P],
                                ident)
                        nc.scalar.copy(
                            out=wprojT[:, :, eb * P:(eb + 1) * P], in_=tp)
                    wv_nat = []
                    for fb in range(F // P):
                        wnat = wload.tile([P, C], F32, tag="wnat")
                        nc.sync.dma_start(
                            out=wnat, in_=w_qkv[fb * P:(fb + 1) * P, :])
                        if fb >= 8:
                            # fp32r matmul inputs must be produced by a
                            # rounding instruction, not a raw DMA
                            wvr = wvnat_pool.tile([P, C], F32R, tag="wv")
                            nc.scalar.copy(out=wvr, in_=wnat)
                            wv_nat.append(wvr)
                        tp = tp_psum.tile([P, CB, P], F32, tag="tp")
                        for cb in range(CB):
                            nc.tensor.transpose(
                                tp[:, cb, :], wnat[:, cb * P:(cb + 1) * P],
                                ident)
                        if fb < 8:
                            nc.vector.tensor_copy(
                                out=wqkT8[:, :, fb * P:(fb + 1) * P], in_=tp)
                        else:
                            nc.scalar.copy(
                                out=wvT[:, :, (fb - 8) * P:(fb - 7) * P],
                                in_=tp)
                    # fused wvpT[c, e] = sum_d w_v[d, c] * wprojT[d, e]
                    for cbp in range(CBP):
                        ps = mm_psum.tile([P, 2, C], F32, tag="ps")
                        for j in range(2):
                            cb = 2 * cbp + j
                            for db in range(CB):
                                nc.tensor.matmul(
                                    ps[:, j, :],
                                    wv_nat[db][:, cb * P:(cb + 1) * P],
                                    wprojT[:, db, :],
                                    start=(db == 0), stop=(db == CB - 1))
                        nc.vector.tensor_copy(
                            out=wvpT8[:, 2 * cbp:2 * cbp + 2, :], in_=ps)

                    for ch in range(n_chunks):
                        n0 = ch * NQ
                        xT = xT_pool.tile([P, CB, NQ], F32R)  # [c, n]
                        for nb in range(nb_total):
                            xn = xnat_pool.tile([P, C], F32, tag="xn")
                            nc.sync.dma_start(
                                out=xn,
                                in_=x[n0 + nb * P:n0 + (nb + 1) * P, :])
                            tp = tp_psum.tile([P, CB, P], F32, tag="tp")
                            for cb in range(CB):
                                nc.tensor.transpose(
                                    tp[:, cb, :],
                                    xn[:, cb * P:(cb + 1) * P], ident)
                            nc.vector.tensor_copy(
                                out=xT[:, :, nb * P:(nb + 1) * P], in_=tp)
                        xT8 = xT8_pool.tile([P, CB, NQ], F8)
                        nc.vector.tensor_copy(out=xT8, in_=f32view(xT))
                        # V natural [n, d] fp32r first: it only needs the
                        # fp32 x^T, so the PE runs it while DVE makes xT8.
                        # nb PAIRS share one 2-bank PSUM tile so every
                        # evacuation is a single 1024-element op.
                        for nbp in range(nb_total // 2):
                            mb = ch * nb_total + 2 * nbp
                            ps = mm_psum.tile([P, 2, C], F32, tag="ps")
                            for j in range(2):
                                nb = 2 * nbp + j
                                for cb in range(CB):
                                    nc.tensor.matmul(
                                        ps[:, j, :],
                                        xT[:, cb, nb * P:(nb + 1) * P],
                                        wvT[:, cb, :],
                                        start=(cb == 0), stop=(cb == CB - 1))
                            nc.vector.tensor_add(
                                out=vb32[:, mb:mb + 2, :], in0=ps,
                                in1=bias_bc2)
                        # Q^T (fb 0..3) / K^T (fb 4..7): fp8 DoubleRow.
                        # All attention operands evacuate via ScalarE so the
                        # phase-2 matmuls depend on a single engine.
                        for fbp in range(4):
                            ps = mm_psum.tile([P, 2, NQ], F32, tag="ps")
                            for j in range(2):
                                fb = 2 * fbp + j
                                for cbp in range(CBP):
                                    nc.tensor.matmul(
                                        ps[:, j, :],
                                        wqkT8[:, 2 * cbp:2 * cbp + 2,
                                              fb * P:(fb + 1) * P],
                                        xT8[:, 2 * cbp:2 * cbp + 2, :],
                                        start=(cbp == 0),
                                        stop=(cbp == CBP - 1),
                                        perf_mode=DR)
                            dst = qT8 if fbp < 2 else kT8
                            d0 = (2 * fbp) % 4
                            nc.scalar.copy(
                                out=dst[:, d0:d0 + 2, n0:n0 + NQ], in_=ps)
                        for nbp in range(nb_total // 2):
                            mb = ch * nb_total + 2 * nbp
                            # VW natural [n, e]: fp8 DoubleRow
                            ps = mm_psum.tile([P, 2, C], F32, tag="ps")
                            for j in range(2):
                                nb = 2 * nbp + j
                                for cbp in range(CBP):
                                    nc.tensor.matmul(
                                        ps[:, j, :],
                                        xT8[:, 2 * cbp:2 * cbp + 2,
                                            nb * P:(nb + 1) * P],
                                        wvpT8[:, 2 * cbp:2 * cbp + 2, :],
                                        start=(cbp == 0),
                                        stop=(cbp == CBP - 1),
                                        perf_mode=DR)
                            nc.scalar.copy(out=vw8[:, mb:mb + 2, :], in_=ps)

            def phase2(strip_den=False):
                with tc.tile_pool(name="pT", bufs=6) as pT_pool, \
                     tc.tile_pool(name="fin", bufs=3) as fin_pool, \
                     tc.tile_pool(name="rs", bufs=2) as rs_pool, \
                     tc.tile_pool(name="st_psum", bufs=2, space="PSUM") as st_psum, \
                     tc.tile_pool(name="pv_psum", bufs=4, space="PSUM") as pv_psum:

                    for ch in range(n_chunks):
                        n0 = ch * NQ
                        pvout = [pv_psum.tile([P, C], F32, tag="pv",
                                              name=f"pv{ch}_{nb}")
                                 for nb in range(nb_total)]
                        acc = rs_pool.tile([P, NQ], F32, tag="acc")

                        def emit_pv(pr, pT):
                            for nb in range(nb_total):
                                nc.tensor.matmul(
                                    pvout[nb],
                                    pT[:, :, nb * P:(nb + 1) * P],
                                    vw8[:, 2 * pr:2 * pr + 2, :],
                                    start=(pr == 0),
                                    stop=(pr == pair_total - 1),
                                    perf_mode=DR)

                        # software-pipelined pair loop: S/exp run one pair
                        # ahead of PV so the PE never waits on the ACT exp
                        pq = []
                        for pr in range(pair_total):
                            stp = st_psum.tile([P, 2, NQ], F32, tag="st")
                            for j in range(2):
                                mb = 2 * pr + j
                                for cbp in range(CBP):
                                    nc.tensor.matmul(
                                        stp[:, j, :],
                                        kT8[:, 2 * cbp:2 * cbp + 2,
                                            mb * P:(mb + 1) * P],
                                        qT8[:, 2 * cbp:2 * cbp + 2,
                                            n0:n0 + NQ],
                                        start=(cbp == 0),
                                        stop=(cbp == CBP - 1),
                                        perf_mode=DR)
                            pT = pT_pool.tile([P, 2, NQ], F8, tag="pT")
                            nc.scalar.activation(
                                out=pT, in_=stp,
                                func=mybir.ActivationFunctionType.Exp,
                                scale=SCALE)
                            pq.append(pT)
                            if strip_den:
                                pass
                            elif pr == 0:
                                nc.vector.tensor_add(
                                    out=acc, in0=pT[:, 0, :],
                                    in1=pT[:, 1, :])
                            else:
                                nc.vector.tensor_add(
                                    out=acc, in0=acc, in1=pT[:, 0, :])
                                nc.vector.tensor_add(
                                    out=acc, in0=acc, in1=pT[:, 1, :])
                            if pr >= 1:
                                emit_pv(pr - 1, pq[pr - 1])
                        emit_pv(pair_total - 1, pq[pair_total - 1])

                        recip_col = rs_pool.tile([P, nb_total], F32,
                                                 tag="recip")
                        if strip_den:
                            nc.vector.memset(recip_col, 1.0)
                        else:
                            # denominators: reuse a dead S-PSUM slot (no
                            # free bank while the 4 pvout accums are live)
                            sums_st = st_psum.tile([P, 2, NQ], F32, tag="st")
                            sums_col = sums_st[:, 0, 0:nb_total]
                            for nb in range(nb_total):
                                nc.tensor.matmul(
                                    sums_col[:, nb:nb + 1],
                                    acc[:, nb * P:(nb + 1) * P], ones_f32,
                                    start=True, stop=True)
                            nc.vector.reciprocal(out=recip_col, in_=sums_col)
                        for nb in range(nb_total):
                            fin = fin_pool.tile([P, C], F32, tag="fin")
                            # fin = pvout * (1/rowsum) + (v + b_proj)
                            nc.vector.scalar_tensor_tensor(
                                out=fin, in0=pvout[nb],
                                scalar=recip_col[:, nb:nb + 1],
                                in1=vb32[:, ch * nb_total + nb, :],
                                op0=mybir.AluOpType.mult,
                                op1=mybir.AluOpType.add)
                            nc.sync.dma_start(
                                out=out[n0 + nb * P:n0 + (nb + 1) * P, :],
                                in_=fin)

            rep_ctx = (tc.For_i(0, hw_loop, 1) if hw_loop
                       else _nullctx())
            if loop_phase in (2, 3):
                phase01()
                with rep_ctx:
                    for _rep in range(reps):
                        phase2(strip_den=(loop_phase == 3))
            else:
                with rep_ctx:
                    for _rep in range(reps):
                        phase01()
                        if loop_phase == 0:
                            phase2()
    _legalize_waits(nc)
    return nc


_PROGRAM_CACHE = {}


def _get_program(n=N_FULL, mm_dt=F32R, attn_dt=F8, reps=1):
    key = (n, mm_dt, attn_dt, reps)
    if key not in _PROGRAM_CACHE:
        _PROGRAM_CACHE[key] = build_program(n, mm_dt, attn_dt, reps=reps)
    return _PROGRAM_CACHE[key]


def kernel(x, w_qkv, w_proj, b_proj):
    from concourse.bass_utils import run_bass_kernel_spmd

    x = np.ascontiguousarray(np.asarray(x, dtype=np.float32))
    w_qkv = np.ascontiguousarray(np.asarray(w_qkv, dtype=np.float32))
    w_proj = np.ascontiguousarray(np.asarray(w_proj, dtype=np.float32))
    b_proj = np.ascontiguousarray(np.asarray(b_proj, dtype=np.float32))
    b, n, c = x.shape
    assert (b, n, c) == (B, N_FULL, C)

    nc = _get_program()
    in_maps = [
        {"x": x[i], "w_qkv": w_qkv, "w_proj": w_proj, "b_proj": b_proj}
        for i in range(B)
    ]
    res = run_bass_kernel_spmd(nc, in_maps, list(range(B)))
    return np.stack([res.results[i]["out"] for i in range(B)], axis=0)


# revision 32
# speedup vs baseline: 1.0178x; 1.0178x over previous
"""Self-contained Trainium2 Bass kernel for single-head T2T attention.

Problem: x:[8,4096,512], w_qkv:[1536,512], w_proj:[512,512], b_proj:[512]
    qkv = x @ w_qkv.T ; q,k,v split
    attn = softmax(q @ k.T / sqrt(512))
    out  = v + (attn @ v) @ w_proj.T + b_proj

Sharding: data-parallel over batch B=8 across the 8 NeuronCores (one
example per core); weights replicated.  No collectives needed.

Numerics: the output is dominated by the v residual (||attn path|| /
||out|| ~ 0.8%), so everything EXCEPT the v residual runs in fp8e4
with DoubleRow perf mode (2 fp8 MACs per PE cell per cycle).  Measured
end-to-end rel err ~6e-4 vs the 2e-2 gate.  V is computed in fp32r
(fp22 multiply) and kept in fp32 for the residual.

Projection fusion: (attn @ v) @ w_proj.T == attn @ (v @ w_proj.T) and
v @ w_proj.T == x @ (w_proj @ w_v).T, so phase 0 computes the fused
weight wvp = (w_proj @ w_v) once on the PE (w_v arrives from DRAM
already in the [d, c] layout needed as the stationary operand, so its
transposed [c, e] layout costs no extra transposes).  The attention PV
matmul then directly produces the PROJECTED output in natural [n, e]
orientation -- no O^T materialization, no separate proj stage, and no
PSUM-evacuation stall between attention and projection.

Per-core dataflow (N=4096, C=512, P=128, NQ=512 chunks):
  phase 0: PE-transpose w_qkv/w_proj; Q/K weight halves to fp8 [c,f];
      V third to fp32r [c,d]; wvpT = w_v.T @ w_proj.T via 16 fp32r
      matmuls -> fp8 [c,e].
  phase 1 (per chunk): stream x, PE-transpose to x^T (fp32), one DVE
      copy to fp8 x^T.  Q^T,K^T via fp8 DoubleRow -> resident SBUF.
      VW = x @ wvp.T via fp8 DoubleRow -> fp8 [m,e] tiles for PV.
      V via fp32r matmuls -> fp32 V+b_proj residual tile (bias folded
      here with a DVE add).
  phase 2 (per chunk, per m-block PAIR): S^T = K.Q^T with DoubleRow,
      both pair halves into one 2-bank PSUM tile; one ScalarE exp
      (scale fused; scores bounded ~|1.1| so no max-subtraction)
      writes the fp8 P^T pair; PV DoubleRow matmuls contract over both
      m-blocks at once: out[nb] += P^T[:, nb-cols].T @ VW-pair,
      accumulating the projected output [n, e] in PSUM.  Softmax
      denominators: DVE accumulates the fp8 P tiles (consistent with
      the PV numerator quantization), tiny N=1 matmuls reduce over
      partitions into a dead S-PSUM slot, and the normalization is a
      per-partition scalar folded into the final fused DVE op:
      fin = pvout * (1/rowsum) + (v + b_proj), then DMA out.
"""

import numpy as np

import concourse.bass as bass
import concourse.mybir as mybir
from concourse.tile import TileContext
from concourse.masks import make_identity

P = 128
B = 8
N_FULL = 4096
C = 512
F = 3 * C
NQ = 512           # query chunk width
CB = C // P        # 4 contraction sub-blocks of 128
CBP = CB // 2      # 2 DoubleRow pairs for a 512 contraction
SCALE = 1.0 / float(np.sqrt(C))
F32 = mybir.dt.float32
F32R = mybir.dt.float32r
BF16 = mybir.dt.bfloat16
F8 = mybir.dt.float8e4
DR = mybir.MatmulPerfMode.DoubleRow


# ---------------------------------------------------------------------------
# Workaround: this container's walrus build accepts at most one sync wait per
# plain instruction (two for EventSemaphore), but Tile's wait assignment can
# attach several.  Post-pass: move excess waits onto injected same-engine
# NOPs placed immediately before the over-subscribed instruction.
# ---------------------------------------------------------------------------
def _legalize_waits(nc):
    for fn in nc.m.functions:
        for bb in fn.blocks:
            insts = bb.instructions
            out = []
            changed = False
            for inst in insts:
                si = inst.sync_info
                waits = list(si.on_wait) if si and si.on_wait else []
                cap = 2 if isinstance(inst, mybir.InstEventSemaphore) else 1
                if len(waits) > cap:
                    keep = waits[:cap]
                    rest = waits[cap:]
                    for i, w in enumerate(rest):
                        nop = mybir.InstNoOp(
                            name=f"{inst.name}-wspill{i}",
                            ins=[], outs=[], engine=inst.engine)
                        nop.sync_info = mybir.SyncInfo(
                            on_wait=[w], on_update=[])
                        nc.register_instruction(nop, overwrite=True)
                        out.append(nop)
                    si.on_wait = keep
                    changed = True
                out.append(inst)
            if changed:
                insts.clear()
                insts.extend(out)


class _nullctx:
    def __enter__(self):
        return None

    def __exit__(self, *a):
        return False


def build_program(n=N_FULL, mm_dt=F32R, attn_dt=F8, reps=1, hw_loop=0,
                  loop_phase=0, xt_bf16=False):
    """Build the per-core Bass program for one [n, C] example.

    mm_dt/attn_dt kept for test.py signature compatibility; the kernel
    is fixed at fp32r (V path) + fp8 DoubleRow (everything else).
    loop_phase (timing probes only): 0 = hw_loop wraps both phases,
    1 = loop wraps phase 0+1 only (no attention), 2 = phase 0+1 run
    once and the loop wraps phase 2 only, 3 = like 2 but with the
    softmax-denominator DVE chain stripped (S/exp/PV floor probe).
    NOTE: DoubleRowSwInterleave was probed and CRASHES the exec unit
    (NRT_EXEC_UNIT_UNRECOVERABLE) on this hardware -- do not use.
    """
    n_chunks = n // NQ
    mb_total = n // P
    pair_total = mb_total // 2
    nb_total = NQ // P

    nc = bass.Bass("TRN2", target_bir_lowering=False,
                   dynamic_dma_scratch_size=8192)
    x = nc.dram_tensor("x", (n, C), F32, kind="ExternalInput")
    w_qkv = nc.dram_tensor("w_qkv", (F, C), F32, kind="ExternalInput")
    w_proj = nc.dram_tensor("w_proj", (C, C), F32, kind="ExternalInput")
    b_proj = nc.dram_tensor("b_proj", (C,), F32, kind="ExternalInput")
    out = nc.dram_tensor("out", (n, C), F32, kind="ExternalOutput")

    def f32view(ap):
        # fp32r storage is fp32 bits; view as fp32 for non-PE ops
        return ap.bitcast(F32) if ap.dtype == F32R else ap

    def r32(ap):
        return ap.bitcast(F32R) if ap.dtype == F32 else ap

    with TileContext(nc) as tc:
        with tc.tile_pool(name="singles", bufs=1) as singles:
            ident = singles.tile([P, P], F32)
            make_identity(nc, ident)
            ones_f32 = singles.tile([P, 1], F32)
            nc.vector.memset(ones_f32, 1.0)
            bias_bc2 = singles.tile([P, 2, C], F32)  # pair-shaped broadcast
            for j in range(2):
                nc.sync.dma_start(
                    out=bias_bc2[:, j, :],
                    in_=b_proj[:].unsqueeze(0).to_broadcast((P, C)))

            # xt_bf16: x^T + V matmuls in bf16 (1-cycle transposes) instead
            # of fp32/f32r; raises final rel err ~7e-4 -> ~2.4e-3 (gate 2e-2)
            xt_dt = BF16 if xt_bf16 else F32R
            if xt_bf16:
                ident_bf = singles.tile([P, P], BF16)
                nc.vector.tensor_copy(out=ident_bf, in_=ident)

            kT8 = singles.tile([P, CB, n], F8)       # K^T: [d, m]
            qT8 = singles.tile([P, CB, n], F8)       # Q^T: [d, n]
            vw8 = singles.tile([P, mb_total, C], F8)  # VW = V @ w_proj.T
            vb32 = singles.tile([P, mb_total, C], F32)  # V + b_proj residual
            wqkT8 = singles.tile([P, CB, 2 * C], F8)    # [c, f] f in [0,1024)
            wvT = singles.tile([P, CB, C], xt_dt)       # [c, d] V third
            wprojT = singles.tile([P, CB, C], F32R)     # [d, e]
            wvpT8 = singles.tile([P, CB, C], F8)        # [c, e] fused V-proj

            def phase01():
                with tc.tile_pool(name="wload", bufs=8) as wload, \
                     tc.tile_pool(name="wvnat", bufs=4) as wvnat_pool, \
                     tc.tile_pool(name="xnat", bufs=6) as xnat_pool, \
                     tc.tile_pool(name="xT", bufs=2) as xT_pool, \
                     tc.tile_pool(name="xT8", bufs=2) as xT8_pool, \
                     tc.tile_pool(name="tp_psum", bufs=3, space="PSUM") as tp_psum, \
                     tc.tile_pool(name="mm_psum", bufs=2, space="PSUM") as mm_psum:

                    # w_proj first (wvp matmuls below need wprojT)
                    for eb in range(C // P):
                        wnat = wload.tile([P, C], F32, tag="wnat")
                        nc.sync.dma_start(
                            out=wnat, in_=w_proj[eb * P:(eb + 1) * P, :])
                        tp = tp_psum.tile([P, CB, P], F32, tag="tp")
                        for db in range(CB):
                            nc.tensor.transpose(
                                tp[:, db, :], wnat[:, db * P:(db + 1) * P],
                                ident)
                        nc.scalar.copy(
                            out=wprojT[:, :, eb * P:(eb + 1) * P], in_=tp)
                    wv_nat = []
                    for fb in range(F // P):
                        wnat = wload.tile([P, C], F32, tag="wnat")
                        nc.sync.dma_start(
                            out=wnat, in_=w_qkv[fb * P:(fb + 1) * P, :])
                        if fb >= 8:
                            # fp32r matmul inputs must be produced by a
                            # rounding instruction, not a raw DMA
                            wvr = wvnat_pool.tile([P, C], F32R, tag="wv")
                            nc.scalar.copy(out=wvr, in_=wnat)
                            wv_nat.append(wvr)
                        tp = tp_psum.tile([P, CB, P], F32, tag="tp")
                        for cb in range(CB):
                            nc.tensor.transpose(
                                tp[:, cb, :], wnat[:, cb * P:(cb + 1) * P],
                                ident)
                        if fb < 8:
                            nc.vector.tensor_copy(
                                out=wqkT8[:, :, fb * P:(fb + 1) * P], in_=tp)
                        else:
                            nc.scalar.copy(
                                out=wvT[:, :, (fb - 8) * P:(fb - 7) * P],
                                in_=tp)
                    # fused wvpT[c, e] = sum_d w_v[d, c] * wprojT[d, e]
                    for cbp in range(CBP):
                        ps = mm_psum.tile([P, 2, C], F32, tag="ps")
                        for j in range(2):
                            cb = 2 * cbp + j
                            for db in range(CB):
                                nc.tensor.matmul(
                                    ps[:, j, :],
                                    wv_nat[db][:, cb * P:(cb + 1) * P],
                                    wprojT[:, db, :],
                                    start=(db == 0), stop=(db == CB - 1))
                        nc.vector.tensor_copy(
                            out=wvpT8[:, 2 * cbp:2 * cbp + 2, :], in_=ps)

                    for ch in range(n_chunks):
                        n0 = ch * NQ
                        xT = xT_pool.tile([P, CB, NQ], xt_dt)  # [c, n]
                        for nb in range(nb_total):
                            xn = xnat_pool.tile([P, C], F32, tag="xn")
                            nc.sync.dma_start(
                                out=xn,
                                in_=x[n0 + nb * P:n0 + (nb + 1) * P, :])
                            if xt_bf16:
                                xn_bf = xnat_pool.tile([P, C], BF16,
                                                       tag="xnb")
                                nc.scalar.copy(out=xn_bf, in_=xn)
                                xn, tr_ident = xn_bf, ident_bf
                            else:
                                tr_ident = ident
                            tp = tp_psum.tile([P, CB, P], xn.dtype,
                                              tag="tp")
                            for cb in range(CB):
                                nc.tensor.transpose(
                                    tp[:, cb, :],
                                    xn[:, cb * P:(cb + 1) * P], tr_ident)
                            nc.vector.tensor_copy(
                                out=xT[:, :, nb * P:(nb + 1) * P], in_=tp)
                        xT8 = xT8_pool.tile([P, CB, NQ], F8)
                        nc.vector.tensor_copy(out=xT8, in_=f32view(xT))
                        # V natural [n, d] fp32r first: it only needs the
                        # fp32 x^T, so the PE runs it while DVE makes xT8.
                        # nb PAIRS share one 2-bank PSUM tile so every
                        # evacuation is a single 1024-element op.
                        for nbp in range(nb_total // 2):
                            mb = ch * nb_total + 2 * nbp
                            ps = mm_psum.tile([P, 2, C], F32, tag="ps")
                            for j in range(2):
                                nb = 2 * nbp + j
                                for cb in range(CB):
                                    nc.tensor.matmul(
                                        ps[:, j, :],
                                        xT[:, cb, nb * P:(nb + 1) * P],
                                        wvT[:, cb, :],
                                        start=(cb == 0), stop=(cb == CB - 1))
                            nc.vector.tensor_add(
                                out=vb32[:, mb:mb + 2, :], in0=ps,
                                in1=bias_bc2)
                        # Q^T (fb 0..3) / K^T (fb 4..7): fp8 DoubleRow.
                        # All attention operands evacuate via ScalarE so the
                        # phase-2 matmuls depend on a single engine.
                        for fbp in range(4):
                            ps = mm_psum.tile([P, 2, NQ], F32, tag="ps")
                            for j in range(2):
                                fb = 2 * fbp + j
                                for cbp in range(CBP):
                                    nc.tensor.matmul(
                                        ps[:, j, :],
                                        wqkT8[:, 2 * cbp:2 * cbp + 2,
                                              fb * P:(fb + 1) * P],
                                        xT8[:, 2 * cbp:2 * cbp + 2, :],
                                        start=(cbp == 0),
                                        stop=(cbp == CBP - 1),
                                        perf_mode=DR)
                            dst = qT8 if fbp < 2 else kT8
                            d0 = (2 * fbp) % 4
                            nc.scalar.copy(
                                out=dst[:, d0:d0 + 2, n0:n0 + NQ], in_=ps)
                        for nbp in range(nb_total // 2):
                            mb = ch * nb_total + 2 * nbp
                            # VW natural [n, e]: fp8 DoubleRow
                            ps = mm_psum.tile([P, 2, C], F32, tag="ps")
                            for j in range(2):
                                nb = 2 * nbp + j
                                for cbp in range(CBP):
                                    nc.tensor.matmul(
                                        ps[:, j, :],
                                        xT8[:, 2 * cbp:2 * cbp + 2,
                                            nb * P:(nb + 1) * P],
                                        wvpT8[:, 2 * cbp:2 * cbp + 2, :],
                                        start=(cbp == 0),
                                        stop=(cbp == CBP - 1),
                                        perf_mode=DR)
                            nc.scalar.copy(out=vw8[:, mb:mb + 2, :], in_=ps)

            def phase2(strip_den=False):
                with tc.tile_pool(name="pT", bufs=6) as pT_pool, \
                     tc.tile_pool(name="fin", bufs=3) as fin_pool, \
                     tc.tile_pool(name="rs", bufs=2) as rs_pool, \
                     tc.tile_pool(name="st_psum", bufs=2, space="PSUM") as st_psum, \
                     tc.tile_pool(name="pv_psum", bufs=4, space="PSUM") as pv_psum:

                    for ch in range(n_chunks):
                        n0 = ch * NQ
                        pvout = [pv_psum.tile([P, C], F32, tag="pv",
                                              name=f"pv{ch}_{nb}")
                                 for nb in range(nb_total)]
                        acc = rs_pool.tile([P, NQ], F32, tag="acc")

                        def emit_pv(pr, pT):
                            for nb in range(nb_total):
                                nc.tensor.matmul(
                                    pvout[nb],
                                    pT[:, :, nb * P:(nb + 1) * P],
                                    vw8[:, 2 * pr:2 * pr + 2, :],
                                    start=(pr == 0),
                                    stop=(pr == pair_total - 1),
                                    perf_mode=DR)

                        # software-pipelined pair loop: S/exp run one pair
                        # ahead of PV so the PE never waits on the ACT exp
                        pq = []
                        for pr in range(pair_total):
                            stp = st_psum.tile([P, 2, NQ], F32, tag="st")
                            for j in range(2):
                                mb = 2 * pr + j
                                for cbp in range(CBP):
                                    nc.tensor.matmul(
                                        stp[:, j, :],
                                        kT8[:, 2 * cbp:2 * cbp + 2,
                                            mb * P:(mb + 1) * P],
                                        qT8[:, 2 * cbp:2 * cbp + 2,
                                            n0:n0 + NQ],
                                        start=(cbp == 0),
                                        stop=(cbp == CBP - 1),
                                        perf_mode=DR)
                            pT = pT_pool.tile([P, 2, NQ], F8, tag="pT")
                            nc.scalar.activation(
                                out=pT, in_=stp,
                                func=mybir.ActivationFunctionType.Exp,
                                scale=SCALE)
                            pq.append(pT)
                            if strip_den:
                                pass
                            elif pr == 0:
                                nc.vector.tensor_add(
                                    out=acc, in0=pT[:, 0, :],
                                    in1=pT[:, 1, :])
                            else:
                                nc.vector.tensor_add(
                                    out=acc, in0=acc, in1=pT[:, 0, :])
                                nc.vector.tensor_add(
                                    out=acc, in0=acc, in1=pT[:, 1, :])
                            if pr >= 1:
                                emit_pv(pr - 1, pq[pr - 1])
                        emit_pv(pair_total - 1, pq[pair_total - 1])

                        recip_col = rs_pool.tile([P, nb_total], F32,
                                                 tag="recip")
                        if strip_den:
                            nc.vector.memset(recip_col, 1.0)
                        else:
                            # denominators: reuse a dead S-PSUM slot (no
                            # free bank while the 4 pvout accums are live)
                            sums_st = st_psum.tile([P, 2, NQ], F32, tag="st")
                            sums_col = sums_st[:, 0, 0:nb_total]
                            for nb in range(nb_total):
                                nc.tensor.matmul(
                                    sums_col[:, nb:nb + 1],
                                    acc[:, nb * P:(nb + 1) * P], ones_f32,
                                    start=True, stop=True)
                            nc.vector.reciprocal(out=recip_col, in_=sums_col)
                        for nb in range(nb_total):
                            fin = fin_pool.tile([P, C], F32, tag="fin")
                            # fin = pvout * (1/rowsum) + (v + b_proj)
                            nc.vector.scalar_tensor_tensor(
                                out=fin, in0=pvout[nb],
                                scalar=recip_col[:, nb:nb + 1],
                                in1=vb32[:, ch * nb_total + nb, :],
                                op0=mybir.AluOpType.mult,
                                op1=mybir.AluOpType.add)
                            nc.sync.dma_start(
                                out=out[n0 + nb * P:n0 + (nb + 1) * P, :],
                                in_=fin)

            rep_ctx = (tc.For_i(0, hw_loop, 1) if hw_loop
                       else _nullctx())
            if loop_phase in (2, 3):
                phase01()
                with rep_ctx:
                    for _rep in range(reps):
                        phase2(strip_den=(loop_phase == 3))
            else:
                with rep_ctx:
                    for _rep in range(reps):
                        phase01()
                        if loop_phase == 0:
                            phase2()
    _legalize_waits(nc)
    return nc


_PROGRAM_CACHE = {}


def _get_program(n=N_FULL, mm_dt=F32R, attn_dt=F8, reps=1):
    key = (n, mm_dt, attn_dt, reps)
    if key not in _PROGRAM_CACHE:
        _PROGRAM_CACHE[key] = build_program(n, mm_dt, attn_dt, reps=reps)
    return _PROGRAM_CACHE[key]


def kernel(x, w_qkv, w_proj, b_proj):
    from concourse.bass_utils import run_bass_kernel_spmd

    x = np.ascontiguousarray(np.asarray(x, dtype=np.float32))
    w_qkv = np.ascontiguousarray(np.asarray(w_qkv, dtype=np.float32))
    w_proj = np.ascontiguousarray(np.asarray(w_proj, dtype=np.float32))
    b_proj = np.ascontiguousarray(np.asarray(b_proj, dtype=np.float32))
    b, n, c = x.shape
    assert (b, n, c) == (B, N_FULL, C)

    nc = _get_program()
    in_maps = [
        {"x": x[i], "w_qkv": w_qkv, "w_proj": w_proj, "b_proj": b_proj}
        for i in range(B)
    ]
    res = run_bass_kernel_spmd(nc, in_maps, list(range(B)))
    return np.stack([res.results[i]["out"] for i in range(B)], axis=0)
